# revision 1
# baseline (speedup 1.0000x reference)
"""Trainium2 Bass kernel for nn_DilationSpconv (3x sparse-conv + BN + ReLU).

Strategy: the voxel set is ~87.6% dense on a (batch, 353, 97) grid, so we
densify on the host and turn the sparse gather-conv into a dense 3x3 conv
implemented with shifted-slice matmuls (no per-element gathers on device).

Sharding: 8 cores = 4 scenes x 2 x-halves. Each core holds its half-scene
plus a 3-column x halo (recompute) -> fully independent cores, no
collectives.

Layout ("interleave-2"): layer tensor XI[128, W]: partition rows 0:64 hold
channels of even grid-sites, rows 64:128 hold channels of odd sites, column
j holds sites (2j, 2j+1). A 128x128 stationary weight block then packs 2x2
(input-parity x output-parity) 64x64 conv-offset blocks, and one matmul
computes 1024 sites' partial outputs with 128-deep contraction. 6 matmuls
cover all 9 offsets of a 3x3 kernel (75% PE utilization). Per-layer phase
shifts (phi = 3,2,1,0) keep the offset runs {g, g+1, g+2} even-aligned so
the 6-matmul covering works for every dx group.

BN+ReLU fused into one ACT op (per-partition scale/bias); occupancy mask
(required so inactive/pad sites stay exactly zero between layers) is one
DVE multiply.
"""

import os
import sys

import numpy as np

for _p in ("/opt/trn_rl_repo", "/opt/pypackages"):
    if os.path.isdir(_p) and _p not in sys.path:
        sys.path.append(_p)

# ---- problem constants (hardcoded, spec: nn_DilationSpconv_7370163880515) ----
N = 120000
C = 64
B = 4
XLIM = 352
YLIM = 96
EPS = 1e-5
NXS = 353  # x grid steps:  x in [-352, 352] step 2
NYS = 97   # y grid steps:  y in [-96, 96] step 2
YP = 100   # padded column height: pad row 0, real rows 1..97, pads 98..99
NCORES = 8
OWN0 = 177          # x-cols owned by even cores (odd cores own 176)
NXL = 184           # local x columns in the per-core dense grid
WCOLS = 512         # matmul window width (PSUM bank = 512 fp32)
LASTW = 384         # narrowed width of the final window (rest is zero pad)
NWIN = 18           # windows per layer
OUTC = NWIN * WCOLS   # 9216 XI columns written per layer
MARG = 64           # lead margin (zero) in XI columns
WBUF = MARG + OUTC + 128  # 9408 total XI columns
PHI = (3, 2, 1, 0)  # storage phase per layer tensor (delta-phi = +1 each layer)
# matmul column-shift offsets v, in order (dx=-1 j=0, dx=-1 j=1, dx=0 ...)
VOFF = (-50, -49, 0, 1, 50, 51)

_CACHE = {}


def _core_geometry(core):
    half = core % 2
    x0 = 0 if half == 0 else OWN0
    own = OWN0 if half == 0 else NXS - OWN0
    xstart = x0 - 4  # local col L maps to global x-step xstart + L
    lo = max(0, x0 - 3)
    hi = min(NXS, x0 + own + 3)
    return x0, own, xstart, lo, hi


def _host_prepare(feat, coor, Ws, scales, biases, np_dt):
    """Build per-core dense interleaved grids, masks, weight stacks, BN vecs."""
    xs = (coor[:, 1].astype(np.int64) + XLIM) // 2  # [0, 353)
    ys = (coor[:, 2].astype(np.int64) + YLIM) // 2  # [0, 97)
    b = coor[:, 0].astype(np.int64)

    xi0 = np.zeros((NCORES, 128, WBUF), np.float32)
    m1 = np.zeros((NCORES, 128, WBUF), np.float32)
    m2 = np.zeros((NCORES, 128, WBUF), np.float32)
    ch = np.arange(C)

    for core in range(NCORES):
        scene = core // 2
        _, _, xstart, lo, hi = _core_geometry(core)
        sel = (b == scene) & (xs >= lo) & (xs < hi)
        L = xs[sel] - xstart
        s = L * YP + ys[sel] + 1
        # layer-0 features at phase 3
        q = s + PHI[0]
        rows = (q & 1) * 64
        cols = MARG + (q >> 1)
        xi0[core, rows[:, None] + ch[None, :], cols[:, None]] = feat[sel]
        # occupancy masks at phases 2 (layer-1 out) and 1 (layer-2 out)
        for mk, phi in ((m1, PHI[1]), (m2, PHI[2])):
            qq = s + phi
            mk[core, ((qq & 1) * 64)[:, None] + ch[None, :],
               (MARG + (qq >> 1))[:, None]] = 1.0

    # weight stacks: per layer, 6 stationaries of [contract 128, out 128]
    def k_of(dxs, dys):
        return 3 * (dxs + 1) + (dys + 1)

    mats = []
    for W in Ws:  # [9, 64, 64] (k, c_in, c_out)
        for dxs in (-1, 0, 1):
            for j in (0, 1):
                M = np.zeros((128, 128), np.float32)
                if j == 0:
                    M[0:64, 0:64] = W[k_of(dxs, -1)]      # A: even-in -> even-out
                    M[64:128, 0:64] = W[k_of(dxs, 0)]     # C: odd-in  -> even-out
                    M[64:128, 64:128] = W[k_of(dxs, -1)]  # D: odd-in  -> odd-out
                else:
                    M[0:64, 0:64] = W[k_of(dxs, 1)]       # A
                    M[0:64, 64:128] = W[k_of(dxs, 0)]     # B: even-in -> odd-out
                    M[64:128, 64:128] = W[k_of(dxs, 1)]   # D
                mats.append(M)
    wstack = np.stack(mats).transpose(1, 0, 2).reshape(128, 18 * 128)

    bnv = np.zeros((128, 8), np.float32)
    for l in range(3):
        bnv[0:64, l] = scales[l]
        bnv[64:128, l] = scales[l]
        bnv[0:64, 3 + l] = biases[l]
        bnv[64:128, 3 + l] = biases[l]

    mask_dt = np.float16 if np_dt == np.float16 else _BF16
    return (xi0.astype(np_dt), m1.astype(mask_dt),
            m2.astype(mask_dt), wstack.astype(np_dt), bnv)


def _build_program(dt_key, loop_n=0, variant="full", psum_bufs=8, nch=8,
                   warmup=10):
    import concourse.tile as tile
    from concourse import bacc, mybir

    f32 = mybir.dt.float32
    if dt_key == "bf16":
        DT = mybir.dt.bfloat16
        mm_cast = None
        BF = mybir.dt.bfloat16
    elif dt_key == "fp16":
        DT = mybir.dt.float16
        mm_cast = None
        BF = mybir.dt.float16
    else:  # f32r: store f32, matmul in float32r (single-pass fp32)
        DT = mybir.dt.float32
        mm_cast = mybir.dt.float32r
        BF = mybir.dt.bfloat16

    nc = bacc.Bacc("TRN2", target_bir_lowering=False, debug=False,
                   num_devices=NCORES)
    xi0_d = nc.dram_tensor("xi0", [128, WBUF], DT, kind="ExternalInput").ap()
    m1_d = nc.dram_tensor("m1", [128, WBUF], BF, kind="ExternalInput").ap()
    m2_d = nc.dram_tensor("m2", [128, WBUF], BF, kind="ExternalInput").ap()
    wts_d = nc.dram_tensor("wts", [128, 18 * 128], DT, kind="ExternalInput").ap()
    bnv_d = nc.dram_tensor("bnv", [128, 8], f32, kind="ExternalInput").ap()
    out_d = nc.dram_tensor("out", [128, OUTC], f32, kind="ExternalOutput").ap()

    Relu = mybir.ActivationFunctionType.Relu
    mult = mybir.AluOpType.mult

    with tile.TileContext(nc) as tc:
        with (
            tc.tile_pool(name="big", bufs=1) as big,
            tc.tile_pool(name="psum", bufs=psum_bufs, space="PSUM") as psump,
            tc.tile_pool(name="tmp", bufs=6) as tmpp,
        ):
            xa = big.tile([128, WBUF], DT)
            xb = big.tile([128, WBUF], DT)
            x3 = big.tile([128, OUTC], f32)
            m1t = big.tile([128, WBUF], BF)
            m2t = big.tile([128, WBUF], BF)
            wt = big.tile([128, 18 * 128], DT)
            bnt = big.tile([128, 8], f32)

            # PE warmup scratch (no DMA dependency, so warmup matmuls start
            # at t~0 and run during the input DMA, bringing the PE
            # clock-gate to 8/8 before the real work starts).
            scr = big.tile([128, 640], DT)

            def body():
                # DMA order matters: the SP HWDGE ring is FIFO, so load what
                # the first matmuls need first (weights, bn, then xi0), masks
                # later (first DVE use trails the first matmul by ~1.5us; m2
                # is not needed until layer 2).
                bnv = bnt
                nc.sync.dma_start(out=bnv, in_=bnv_d)
                nc.sync.dma_start(out=wt, in_=wts_d)
                chw = WBUF // nch
                # All input DMAs on the SP HWDGE ring: routing masks through
                # the ACT ring measured ~25us slower (mask DMA descriptor
                # issue blocks activation dispatch on the ACT sequencer).
                mask_eng = nc.scalar if variant == "dma2ring" else nc.sync
                # leading xi0 chunks small so the first matmul starts early
                edges = [0, 320, 640, 1176]
                while edges[-1] < WBUF:
                    edges.append(min(WBUF, edges[-1] + chw))
                for a, bnd in zip(edges[:-1], edges[1:]):
                    nc.sync.dma_start(out=xa[:, a:bnd], in_=xi0_d[:, a:bnd])
                for t, d in ((m1t, m1_d), (m2t, m2_d)):
                    for i in range(nch):
                        mask_eng.dma_start(out=t[:, i * chw:(i + 1) * chw],
                                           in_=d[:, i * chw:(i + 1) * chw])
                # xb margins must read as zero for layer-2's shifted slices
                nc.vector.memset(xb[:, 0:MARG], 0.0)
                nc.vector.memset(xb[:, MARG + OUTC:WBUF], 0.0)

                nc.vector.memset(scr, 0.0)
                for wu in range(warmup):
                    wps = psump.tile([128, WCOLS], f32, tag="ps")
                    nc.tensor.matmul(wps, scr[:, 0:128], scr[:, 128:640],
                                     start=True, stop=True)

                if variant == "mmreuse":
                    # Stationary-reuse ordering: same lhsT streams CH windows
                    # back-to-back (tests LDWEIGHTS elision/overlap on HW).
                    CH = 4
                    for l in range(3):
                        for w0 in range(0, NWIN, CH):
                            ch_n = min(CH, NWIN - w0)
                            pss = [psump.tile([128, WCOLS], f32, tag="ps",
                                              name=f"ps_{l}_{w0}_{j}")
                                   for j in range(ch_n)]
                            for i, v in enumerate(VOFF):
                                lhsT = wt[:, (6 * l + i) * 128:
                                          (6 * l + i + 1) * 128]
                                for j in range(ch_n):
                                    base = MARG + (w0 + j) * WCOLS
                                    nc.tensor.matmul(
                                        pss[j], lhsT,
                                        xa[:, base + v:base + v + WCOLS],
                                        start=(i == 0), stop=(i == 5))
                    nc.scalar.activation(x3[:, 0:WCOLS], pss[0], Relu,
                                         bias=bnv[:, 3:4], scale=bnv[:, 0:1])
                    nc.sync.dma_start(out=out_d[:, 0:WCOLS],
                                      in_=x3[:, 0:WCOLS])
                    return

                if variant in ("mm", "mmsame", "mmhalf"):
                    # PE-throughput isolation: all matmuls read xa, results
                    # discarded (one token ACT+DMA at the end for liveness).
                    ncols = WCOLS // 2 if variant == "mmhalf" else WCOLS
                    for l in range(3):
                        for w in range(NWIN):
                            ps = psump.tile([128, WCOLS], f32, tag="ps")
                            base = MARG + w * WCOLS
                            for i, v in enumerate(VOFF):
                                wi = 0 if variant == "mmsame" else 6 * l + i
                                lhsT = wt[:, wi * 128:(wi + 1) * 128]
                                rhs = xa[:, base + v:base + v + ncols]
                                nc.tensor.matmul(ps[:, 0:ncols], lhsT, rhs,
                                                 start=(i == 0), stop=(i == 5))
                    nc.scalar.activation(x3[:, 0:WCOLS], ps, Relu,
                                         bias=bnv[:, 3:4], scale=bnv[:, 0:1])
                    nc.sync.dma_start(out=out_d[:, 0:WCOLS],
                                      in_=x3[:, 0:WCOLS])
                    return

                layers = ((xa, xb, m1t, 0), (xb, xa, m2t, 1), (xa, x3, None, 2))
                if variant == "fullreuse":
                    CH = 4
                    for xin, xout, mt, l in layers:
                        for w0 in range(0, NWIN, CH):
                            ch_n = min(CH, NWIN - w0)
                            pss = [psump.tile([128, WCOLS], f32, tag="ps",
                                              name=f"fr_{l}_{w0}_{j}")
                                   for j in range(ch_n)]
                            for i, v in enumerate(VOFF):
                                lhsT = wt[:, (6 * l + i) * 128:
                                          (6 * l + i + 1) * 128]
                                for j in range(ch_n):
                                    base = MARG + (w0 + j) * WCOLS
                                    nc.tensor.matmul(
                                        pss[j], lhsT,
                                        xin[:, base + v:base + v + WCOLS],
                                        start=(i == 0), stop=(i == 5))
                            sc = bnv[:, l:l + 1]
                            bi = bnv[:, 3 + l:4 + l]
                            for j in range(ch_n):
                                w = w0 + j
                                base = MARG + w * WCOLS
                                if mt is not None:
                                    tm = tmpp.tile([128, WCOLS], DT,
                                                   tag="tm",
                                                   name=f"tm_{l}_{w}")
                                    nc.scalar.activation(tm, pss[j], Relu,
                                                         bias=bi, scale=sc)
                                    nc.vector.tensor_tensor(
                                        out=xout[:, base:base + WCOLS],
                                        in0=tm,
                                        in1=mt[:, base:base + WCOLS],
                                        op=mult)
                                else:
                                    dst = xout[:, w * WCOLS:(w + 1) * WCOLS]
                                    nc.scalar.activation(dst, pss[j], Relu,
                                                         bias=bi, scale=sc)
                                    nc.sync.dma_start(
                                        out=out_d[:, w * WCOLS:
                                                  (w + 1) * WCOLS],
                                        in_=dst)
                    return

                for xin, xout, mt, l in layers:
                    if variant == "nodve":
                        mt = None
                    for w in range(NWIN):
                        wc = WCOLS
                        ps = psump.tile([128, WCOLS], f32, tag="ps")
                        base = MARG + w * WCOLS
                        for i, v in enumerate(VOFF):
                            lhsT = wt[:, (6 * l + i) * 128:(6 * l + i + 1) * 128]
                            rhs = xin[:, base + v:base + v + wc]
                            if mm_cast is not None:
                                lhsT = lhsT.bitcast(mm_cast)
                                rhs = rhs.bitcast(mm_cast)
                            nc.tensor.matmul(ps[:, 0:wc], lhsT, rhs,
                                             start=(i == 0), stop=(i == 5))
                        sc = bnv[:, l:l + 1]
                        bi = bnv[:, 3 + l:4 + l]
                        if mt is not None:
                            tm = tmpp.tile([128, WCOLS], DT)
                            nc.scalar.activation(tm[:, 0:wc], ps[:, 0:wc],
                                                 Relu, bias=bi, scale=sc)
                            nc.vector.tensor_tensor(
                                out=xout[:, base:base + wc], in0=tm[:, 0:wc],
                                in1=mt[:, base:base + wc], op=mult)
                        elif l < 2:
                            nc.scalar.activation(
                                xout[:, base:base + wc], ps[:, 0:wc], Relu,
                                bias=bi, scale=sc)
                        else:
                            dst = xout[:, w * WCOLS:w * WCOLS + wc]
                            nc.scalar.activation(dst, ps[:, 0:wc], Relu,
                                                 bias=bi, scale=sc)
                            nc.sync.dma_start(
                                out=out_d[:, w * WCOLS:w * WCOLS + wc],
                                in_=dst)

            if loop_n > 0:
                with tc.For_i(0, loop_n, 1):
                    body()
            else:
                body()
    nc.compile()
    return nc


def _get_np_dt(dt_key):
    if dt_key == "bf16":
        import ml_dtypes
        return ml_dtypes.bfloat16
    if dt_key == "fp16":
        return np.float16
    return np.float32


def kernel(feat, coor, kin_idx,
           W1, g1, b1, m1, v1,
           W2, g2, b2, m2, v2,
           W3, g3, b3, m3, v3):
    from concourse import bass_utils

    dt_key = os.environ.get("KERNEL_DT", "fp16")
    np_dt = _get_np_dt(dt_key)

    feat = np.asarray(feat, np.float32)
    coor = np.asarray(coor)
    Ws = [np.asarray(W, np.float32) for W in (W1, W2, W3)]
    scales, biases = [], []
    for g, bb, mm, vv in ((g1, b1, m1, v1), (g2, b2, m2, v2), (g3, b3, m3, v3)):
        s = np.asarray(g, np.float32) / np.sqrt(np.asarray(vv, np.float32) + EPS)
        scales.append(s)
        biases.append(np.asarray(bb, np.float32) - np.asarray(mm, np.float32) * s)

    xi0, m1g, m2g, wstack, bnv = _host_prepare(feat, coor, Ws, scales, biases,
                                               np_dt)

    if dt_key not in _CACHE:
        _CACHE[dt_key] = _build_program(dt_key)
    nc = _CACHE[dt_key]

    in_maps = [
        {"xi0": np.ascontiguousarray(xi0[c]),
         "m1": np.ascontiguousarray(m1g[c]),
         "m2": np.ascontiguousarray(m2g[c]),
         "wts": wstack, "bnv": bnv}
        for c in range(NCORES)
    ]
    res = None
    for attempt in range(3):
        try:
            res = bass_utils.run_bass_kernel_spmd(
                nc, in_maps, core_ids=list(range(NCORES)))
            break
        except Exception:
            if attempt == 2:
                raise
            import time
            time.sleep(5)
    grids = np.stack([r["out"] for r in res.results])  # [8, 128, 9216]
    grids = grids.reshape(NCORES, 2, 64, OUTC)

    # gather per-voxel rows from the owning core's grid (phase 0)
    xs = (coor[:, 1].astype(np.int64) + XLIM) // 2
    ys = (coor[:, 2].astype(np.int64) + YLIM) // 2
    b = coor[:, 0].astype(np.int64)
    half = (xs >= OWN0).astype(np.int64)
    core = 2 * b + half
    xstart = np.where(half == 0, -4, OWN0 - 4)
    s = (xs - xstart) * YP + ys + 1
    out = grids[core, s & 1, :, s >> 1].astype(np.float32)  # [N, 64]

    xy_ok = ((coor[:, 1] > -XLIM) & (coor[:, 1] <= XLIM)
             & (coor[:, 2] > -YLIM) & (coor[:, 2] <= YLIM))
    out *= xy_ok[:, None].astype(np.float32)
    return out


_BF16 = None


def _init_bf16():
    global _BF16
    import ml_dtypes
    _BF16 = ml_dtypes.bfloat16


_init_bf16()



# revision 14
# speedup vs baseline: 1.4245x; 1.4245x over previous
"""Trainium2 Bass kernel for nn_DilationSpconv (3x sparse-conv + BN + ReLU).

Strategy: the voxel set is ~87.6% dense on a (batch, 353, 97) grid, so we
densify on the host and turn the sparse gather-conv into a dense 3x3 conv
implemented with shifted-slice matmuls (no per-element gathers on device).

Sharding: 8 cores = 4 scenes x 2 x-halves. Each core holds its half-scene
plus a 3-column x halo (recompute) -> fully independent cores, no
collectives.

Layout ("interleave-2"): layer tensor XI[128, W]: partition rows 0:64 hold
channels of even grid-sites, rows 64:128 hold channels of odd sites, column
j holds sites (2j, 2j+1). A 128x128 stationary weight block then packs 2x2
(input-parity x output-parity) 64x64 conv-offset blocks, and one matmul
computes 1024 sites' partial outputs with 128-deep contraction. 6 matmuls
cover all 9 offsets of a 3x3 kernel (75% PE utilization). Per-layer phase
shifts (phi = 3,2,1,0) keep the offset runs {g, g+1, g+2} even-aligned so
the 6-matmul covering works for every dx group.

BN+ReLU fused into one ACT op (per-partition scale/bias); occupancy mask
(required so inactive/pad sites stay exactly zero between layers) is one
DVE multiply.
"""

import os
import sys

import numpy as np

for _p in ("/opt/trn_rl_repo", "/opt/pypackages"):
    if os.path.isdir(_p) and _p not in sys.path:
        sys.path.append(_p)

# ---- problem constants (hardcoded, spec: nn_DilationSpconv_7370163880515) ----
N = 120000
C = 64
B = 4
XLIM = 352
YLIM = 96
EPS = 1e-5
NXS = 353  # x grid steps:  x in [-352, 352] step 2
NYS = 97   # y grid steps:  y in [-96, 96] step 2
YP = 100   # padded column height: pad row 0, real rows 1..97, pads 98..99
NCORES = 8
OWN0 = 177          # x-cols owned by even cores (odd cores own 176)
NXL = 184           # local x columns in the per-core dense grid
WCOLS = 512         # matmul window width (PSUM bank = 512 fp32)
LASTW = 384         # narrowed width of the final window (rest is zero pad)
NWIN = 18           # windows per layer
OUTC = NWIN * WCOLS   # 9216 XI columns written per layer
MARG = 64           # lead margin (zero) in XI columns
WBUF = MARG + OUTC + 128  # 9408 total XI columns
PHI = (3, 2, 1, 0)  # storage phase per layer tensor (delta-phi = +1 each layer)
# matmul column-shift offsets v, in order (dx=-1 j=0, dx=-1 j=1, dx=0 ...)
VOFF = (-50, -49, 0, 1, 50, 51)
# Per-layer computed column ranges [c0, c1) rel MARG (halo shrinks one x-col
# = 50 XI cols per layer; ranges nest exactly: layer l+1 reads cols
# [c0-50, c1+51) of layer l's output, all written or zeroed).
LAYER_CR = ((101, 9150), (151, 9100), (200, 9049))

_CACHE = {}


def _core_geometry(core):
    half = core % 2
    x0 = 0 if half == 0 else OWN0
    own = OWN0 if half == 0 else NXS - OWN0
    xstart = x0 - 4  # local col L maps to global x-step xstart + L
    lo = max(0, x0 - 3)
    hi = min(NXS, x0 + own + 3)
    return x0, own, xstart, lo, hi


def _host_prepare(feat, coor, Ws, scales, biases, np_dt):
    """Build per-core dense interleaved grids, masks, weight stacks, BN vecs."""
    xs = (coor[:, 1].astype(np.int64) + XLIM) // 2  # [0, 353)
    ys = (coor[:, 2].astype(np.int64) + YLIM) // 2  # [0, 97)
    b = coor[:, 0].astype(np.int64)

    xi0 = np.zeros((NCORES, 128, WBUF), np.float32)
    m1 = np.zeros((NCORES, 128, WBUF), np.float32)
    m2 = np.zeros((NCORES, 128, WBUF), np.float32)
    ch = np.arange(C)

    for core in range(NCORES):
        scene = core // 2
        _, _, xstart, lo, hi = _core_geometry(core)
        sel = (b == scene) & (xs >= lo) & (xs < hi)
        L = xs[sel] - xstart
        s = L * YP + ys[sel] + 1
        # layer-0 features at phase 3
        q = s + PHI[0]
        rows = (q & 1) * 64
        cols = MARG + (q >> 1)
        xi0[core, rows[:, None] + ch[None, :], cols[:, None]] = feat[sel]
        # occupancy masks at phases 2 (layer-1 out) and 1 (layer-2 out)
        for mk, phi in ((m1, PHI[1]), (m2, PHI[2])):
            qq = s + phi
            mk[core, ((qq & 1) * 64)[:, None] + ch[None, :],
               (MARG + (qq >> 1))[:, None]] = 1.0

    # weight stacks: per layer, 6 stationaries of [contract 128, out 128]
    def k_of(dxs, dys):
        return 3 * (dxs + 1) + (dys + 1)

    mats = []
    for W in Ws:  # [9, 64, 64] (k, c_in, c_out)
        for dxs in (-1, 0, 1):
            for j in (0, 1):
                M = np.zeros((128, 128), np.float32)
                if j == 0:
                    M[0:64, 0:64] = W[k_of(dxs, -1)]      # A: even-in -> even-out
                    M[64:128, 0:64] = W[k_of(dxs, 0)]     # C: odd-in  -> even-out
                    M[64:128, 64:128] = W[k_of(dxs, -1)]  # D: odd-in  -> odd-out
                else:
                    M[0:64, 0:64] = W[k_of(dxs, 1)]       # A
                    M[0:64, 64:128] = W[k_of(dxs, 0)]     # B: even-in -> odd-out
                    M[64:128, 64:128] = W[k_of(dxs, 1)]   # D
                mats.append(M)
    wstack = np.stack(mats).transpose(1, 0, 2).reshape(128, 18 * 128)

    bnv = np.zeros((128, 8), np.float32)
    for l in range(3):
        bnv[0:64, l] = scales[l]
        bnv[64:128, l] = scales[l]
        bnv[0:64, 3 + l] = biases[l]
        bnv[64:128, 3 + l] = biases[l]

    mask_dt = np.float16 if np_dt == np.float16 else _BF16
    return (xi0.astype(np_dt), m1.astype(mask_dt),
            m2.astype(mask_dt), wstack.astype(np_dt), bnv)


def _windows(c0, c1):
    ws = []
    b = c0
    while b < c1:
        wc = min(WCOLS, c1 - b)
        ws.append((b, wc))
        b += wc
    return ws


def _build_program(dt_key, loop_n=0, variant="full", psum_bufs=8, nch=8,
                   warmup=5):
    import concourse.tile as tile
    from concourse import bacc, mybir

    f32 = mybir.dt.float32
    if dt_key == "bf16":
        DT = mybir.dt.bfloat16
        mm_cast = None
        BF = mybir.dt.bfloat16
    elif dt_key == "fp16":
        DT = mybir.dt.float16
        mm_cast = None
        BF = mybir.dt.float16
    else:  # f32r: store f32, matmul in float32r (single-pass fp32)
        DT = mybir.dt.float32
        mm_cast = mybir.dt.float32r
        BF = mybir.dt.bfloat16

    nc = bacc.Bacc("TRN2", target_bir_lowering=False, debug=False,
                   num_devices=NCORES)
    xi0_d = nc.dram_tensor("xi0", [128, WBUF], DT, kind="ExternalInput").ap()
    m1_d = nc.dram_tensor("m1", [128, WBUF], BF, kind="ExternalInput").ap()
    m2_d = nc.dram_tensor("m2", [128, WBUF], BF, kind="ExternalInput").ap()
    wts_d = nc.dram_tensor("wts", [128, 18 * 128], DT, kind="ExternalInput").ap()
    bnv_d = nc.dram_tensor("bnv", [128, 8], f32, kind="ExternalInput").ap()
    out_dt = DT if dt_key == "fp16" else f32
    out_d = nc.dram_tensor("out", [128, OUTC], out_dt, kind="ExternalOutput").ap()

    Relu = mybir.ActivationFunctionType.Relu
    mult = mybir.AluOpType.mult

    with tile.TileContext(nc) as tc:
        with (
            tc.tile_pool(name="big", bufs=1) as big,
            tc.tile_pool(name="psum", bufs=psum_bufs, space="PSUM") as psump,
            tc.tile_pool(name="tmp", bufs=6) as tmpp,
        ):
            xa = big.tile([128, WBUF], DT)
            xb = big.tile([128, WBUF], DT)
            x3 = big.tile([128, OUTC], out_dt)
            m1t = big.tile([128, WBUF], BF)
            m2t = big.tile([128, WBUF], BF)
            wt = big.tile([128, 18 * 128], DT)
            bnt = big.tile([128, 8], f32)

            # PE warmup scratch (no DMA dependency, so warmup matmuls start
            # at t~0 and run during the input DMA, bringing the PE
            # clock-gate to 8/8 before the real work starts).
            scr = big.tile([128, 640], DT)

            bnv = bnt

            def init_once():
                # scr first: the warmup matmuls wait on it. xb cols outside
                # layer-1's written range [101, 9150) must read as zero for
                # layer-2's shifted slices; layer 1 never writes these cols,
                # so one memset persists across For_i iterations.
                nc.vector.memset(scr, 0.0)
                nc.vector.memset(xb[:, 0:MARG + 101], 0.0)
                nc.vector.memset(xb[:, MARG + 9150:WBUF], 0.0)

            def body_dma():
                # DMA order matters: the SP HWDGE ring is FIFO and each
                # dma_start costs ~625ns of descriptor issue, so load what
                # the first consumers need first: layer-1 stationaries, the
                # first xi0 chunk (covers window-0 reads [51, 664)), bn
                # vector, then a small leading m1 chunk so DVE unblocks
                # early, then the xi0 bulk, remaining weights, mask bulk.
                nc.sync.dma_start(out=wt[:, 0:768], in_=wts_d[:, 0:768])
                nc.sync.dma_start(out=xa[:, 0:736], in_=xi0_d[:, 0:736])
                nc.sync.dma_start(out=xa[:, 736:1960], in_=xi0_d[:, 736:1960])
                nc.sync.dma_start(out=bnv, in_=bnv_d)
                nc.sync.dma_start(out=m1t[:, 0:1152], in_=m1_d[:, 0:1152])
                edges = [1960, 3200, 4440, 5680, 6920, 8160, WBUF]
                for a, bnd in zip(edges[:-1], edges[1:]):
                    nc.sync.dma_start(out=xa[:, a:bnd], in_=xi0_d[:, a:bnd])
                nc.sync.dma_start(out=wt[:, 768:2304], in_=wts_d[:, 768:2304])
                for a, bnd in ((1152, 3456), (3456, 6432), (6432, WBUF)):
                    nc.sync.dma_start(out=m1t[:, a:bnd], in_=m1_d[:, a:bnd])
                for a, bnd in ((0, 2352), (2352, 4704), (4704, 7056),
                               (7056, WBUF)):
                    nc.sync.dma_start(out=m2t[:, a:bnd], in_=m2_d[:, a:bnd])

            def body():
                if variant in ("full", "nodve"):
                    body_dma()
                else:
                    # isolation variants keep the original DMA pattern
                    nc.sync.dma_start(out=bnv, in_=bnv_d)
                    nc.sync.dma_start(out=wt, in_=wts_d)
                    chw = WBUF // nch
                    mask_eng = nc.scalar if variant == "dma2ring" else nc.sync
                    edges = [0, 320, 640, 1176]
                    while edges[-1] < WBUF:
                        edges.append(min(WBUF, edges[-1] + chw))
                    for a, bnd in zip(edges[:-1], edges[1:]):
                        nc.sync.dma_start(out=xa[:, a:bnd], in_=xi0_d[:, a:bnd])
                    for t, d in ((m1t, m1_d), (m2t, m2_d)):
                        for i in range(nch):
                            mask_eng.dma_start(out=t[:, i * chw:(i + 1) * chw],
                                               in_=d[:, i * chw:(i + 1) * chw])

                for wu in range(warmup):
                    wps = psump.tile([128, WCOLS], f32, tag="ps")
                    nc.tensor.matmul(wps, scr[:, 0:128], scr[:, 128:640],
                                     start=True, stop=True)

                if variant == "mmreuse":
                    # Stationary-reuse ordering: same lhsT streams CH windows
                    # back-to-back (tests LDWEIGHTS elision/overlap on HW).
                    CH = 4
                    for l in range(3):
                        for w0 in range(0, NWIN, CH):
                            ch_n = min(CH, NWIN - w0)
                            pss = [psump.tile([128, WCOLS], f32, tag="ps",
                                              name=f"ps_{l}_{w0}_{j}")
                                   for j in range(ch_n)]
                            for i, v in enumerate(VOFF):
                                lhsT = wt[:, (6 * l + i) * 128:
                                          (6 * l + i + 1) * 128]
                                for j in range(ch_n):
                                    base = MARG + (w0 + j) * WCOLS
                                    nc.tensor.matmul(
                                        pss[j], lhsT,
                                        xa[:, base + v:base + v + WCOLS],
                                        start=(i == 0), stop=(i == 5))
                    nc.scalar.activation(x3[:, 0:WCOLS], pss[0], Relu,
                                         bias=bnv[:, 3:4], scale=bnv[:, 0:1])
                    nc.sync.dma_start(out=out_d[:, 0:WCOLS],
                                      in_=x3[:, 0:WCOLS])
                    return

                if variant in ("mm", "mmsame", "mmhalf"):
                    # PE-throughput isolation: all matmuls read xa, results
                    # discarded (one token ACT+DMA at the end for liveness).
                    ncols = WCOLS // 2 if variant == "mmhalf" else WCOLS
                    for l in range(3):
                        for w in range(NWIN):
                            ps = psump.tile([128, WCOLS], f32, tag="ps")
                            base = MARG + w * WCOLS
                            for i, v in enumerate(VOFF):
                                wi = 0 if variant == "mmsame" else 6 * l + i
                                lhsT = wt[:, wi * 128:(wi + 1) * 128]
                                rhs = xa[:, base + v:base + v + ncols]
                                nc.tensor.matmul(ps[:, 0:ncols], lhsT, rhs,
                                                 start=(i == 0), stop=(i == 5))
                    nc.scalar.activation(x3[:, 0:WCOLS], ps, Relu,
                                         bias=bnv[:, 3:4], scale=bnv[:, 0:1])
                    nc.sync.dma_start(out=out_d[:, 0:WCOLS],
                                      in_=x3[:, 0:WCOLS])
                    return

                layers = ((xa, xb, m1t, 0), (xb, xa, m2t, 1), (xa, x3, None, 2))
                if variant == "fullreuse":
                    CH = 4
                    for xin, xout, mt, l in layers:
                        for w0 in range(0, NWIN, CH):
                            ch_n = min(CH, NWIN - w0)
                            pss = [psump.tile([128, WCOLS], f32, tag="ps",
                                              name=f"fr_{l}_{w0}_{j}")
                                   for j in range(ch_n)]
                            for i, v in enumerate(VOFF):
                                lhsT = wt[:, (6 * l + i) * 128:
                                          (6 * l + i + 1) * 128]
                                for j in range(ch_n):
                                    base = MARG + (w0 + j) * WCOLS
                                    nc.tensor.matmul(
                                        pss[j], lhsT,
                                        xin[:, base + v:base + v + WCOLS],
                                        start=(i == 0), stop=(i == 5))
                            sc = bnv[:, l:l + 1]
                            bi = bnv[:, 3 + l:4 + l]
                            for j in range(ch_n):
                                w = w0 + j
                                base = MARG + w * WCOLS
                                if mt is not None:
                                    tm = tmpp.tile([128, WCOLS], DT,
                                                   tag="tm",
                                                   name=f"tm_{l}_{w}")
                                    nc.scalar.activation(tm, pss[j], Relu,
                                                         bias=bi, scale=sc)
                                    nc.vector.tensor_tensor(
                                        out=xout[:, base:base + WCOLS],
                                        in0=tm,
                                        in1=mt[:, base:base + WCOLS],
                                        op=mult)
                                else:
                                    dst = xout[:, w * WCOLS:(w + 1) * WCOLS]
                                    nc.scalar.activation(dst, pss[j], Relu,
                                                         bias=bi, scale=sc)
                                    nc.sync.dma_start(
                                        out=out_d[:, w * WCOLS:
                                                  (w + 1) * WCOLS],
                                        in_=dst)
                    return

                for xin, xout, mt, l in layers:
                    if variant == "nodve":
                        mt = None
                    c0, c1 = LAYER_CR[l]
                    pend = None  # pending out-DMA range [a, b)
                    for b0, wc in _windows(c0, c1):
                        ps = psump.tile([128, WCOLS], f32, tag="ps")
                        base = MARG + b0
                        for i, v in enumerate(VOFF):
                            lhsT = wt[:, (6 * l + i) * 128:(6 * l + i + 1) * 128]
                            rhs = xin[:, base + v:base + v + wc]
                            if mm_cast is not None:
                                lhsT = lhsT.bitcast(mm_cast)
                                rhs = rhs.bitcast(mm_cast)
                            nc.tensor.matmul(ps[:, 0:wc], lhsT, rhs,
                                             start=(i == 0), stop=(i == 5))
                        sc = bnv[:, l:l + 1]
                        bi = bnv[:, 3 + l:4 + l]
                        if mt is not None:
                            tm = tmpp.tile([128, WCOLS], DT)
                            nc.scalar.activation(tm[:, 0:wc], ps[:, 0:wc],
                                                 Relu, bias=bi, scale=sc)
                            nc.vector.tensor_tensor(
                                out=xout[:, base:base + wc], in0=tm[:, 0:wc],
                                in1=mt[:, base:base + wc], op=mult)
                        elif l < 2:
                            nc.scalar.activation(
                                xout[:, base:base + wc], ps[:, 0:wc], Relu,
                                bias=bi, scale=sc)
                        else:
                            dst = xout[:, b0:b0 + wc]
                            nc.scalar.activation(dst, ps[:, 0:wc], Relu,
                                                 bias=bi, scale=sc)
                            nc.sync.dma_start(out=out_d[:, b0:b0 + wc],
                                              in_=dst)
                    if l == 0:
                        # xa col 150 holds xi0 halo data layer 2 won't
                        # overwrite ([151, 9100)); layer 3 reads it as the
                        # pad sites 299/300 and needs zero.
                        nc.vector.memset(xa[:, MARG + 150:MARG + 151], 0.0)

            init_once()
            if loop_n > 0:
                with tc.For_i(0, loop_n, 1):
                    body()
            else:
                body()
    nc.compile()
    return nc


def _get_np_dt(dt_key):
    if dt_key == "bf16":
        import ml_dtypes
        return ml_dtypes.bfloat16
    if dt_key == "fp16":
        return np.float16
    return np.float32


def kernel(feat, coor, kin_idx,
           W1, g1, b1, m1, v1,
           W2, g2, b2, m2, v2,
           W3, g3, b3, m3, v3):
    from concourse import bass_utils

    dt_key = os.environ.get("KERNEL_DT", "fp16")
    np_dt = _get_np_dt(dt_key)

    feat = np.asarray(feat, np.float32)
    coor = np.asarray(coor)
    Ws = [np.asarray(W, np.float32) for W in (W1, W2, W3)]
    scales, biases = [], []
    for g, bb, mm, vv in ((g1, b1, m1, v1), (g2, b2, m2, v2), (g3, b3, m3, v3)):
        s = np.asarray(g, np.float32) / np.sqrt(np.asarray(vv, np.float32) + EPS)
        scales.append(s)
        biases.append(np.asarray(bb, np.float32) - np.asarray(mm, np.float32) * s)

    xi0, m1g, m2g, wstack, bnv = _host_prepare(feat, coor, Ws, scales, biases,
                                               np_dt)

    if dt_key not in _CACHE:
        _CACHE[dt_key] = _build_program(dt_key)
    nc = _CACHE[dt_key]

    in_maps = [
        {"xi0": np.ascontiguousarray(xi0[c]),
         "m1": np.ascontiguousarray(m1g[c]),
         "m2": np.ascontiguousarray(m2g[c]),
         "wts": wstack, "bnv": bnv}
        for c in range(NCORES)
    ]
    res = None
    for attempt in range(3):
        try:
            res = bass_utils.run_bass_kernel_spmd(
                nc, in_maps, core_ids=list(range(NCORES)))
            break
        except Exception:
            if attempt == 2:
                raise
            import time
            time.sleep(5)
    grids = np.stack([r["out"] for r in res.results])  # [8, 128, 9216]
    grids = grids.reshape(NCORES, 2, 64, OUTC)

    # gather per-voxel rows from the owning core's grid (phase 0)
    xs = (coor[:, 1].astype(np.int64) + XLIM) // 2
    ys = (coor[:, 2].astype(np.int64) + YLIM) // 2
    b = coor[:, 0].astype(np.int64)
    half = (xs >= OWN0).astype(np.int64)
    core = 2 * b + half
    xstart = np.where(half == 0, -4, OWN0 - 4)
    s = (xs - xstart) * YP + ys + 1
    out = grids[core, s & 1, :, s >> 1].astype(np.float32)  # [N, 64]

    xy_ok = ((coor[:, 1] > -XLIM) & (coor[:, 1] <= XLIM)
             & (coor[:, 2] > -YLIM) & (coor[:, 2] <= YLIM))
    out *= xy_ok[:, None].astype(np.float32)
    return out


_BF16 = None


def _init_bf16():
    global _BF16
    import ml_dtypes
    _BF16 = ml_dtypes.bfloat16


_init_bf16()

